# revision 18
# baseline (speedup 1.0000x reference)
"""GTN-Rec on 8 TRN2 NeuronCores.

Strategy (sharding over the item dim N=2000, 250 columns per core):
  - a0/b0/a2 (softmax-mixed adjacency combos) computed per-core from the
    local A column shard on the vector engine, fp16.
  - Transposed matmul chain y1T/y2T/y3T = (a0|b0|a2)^T-shard @ prev, with
    fp16 AllGather of the [250, 1920] activation shards between stages.
  - enc/lin layer in fp32 (values ~1e7 need precision), ReduceScatter of the
    [128, 1920] basket partial into batch shards (8 batches per core).
  - LSTM tail runs batch-sharded in a transposed [U, batch] layout so h
    needs no per-step transpose; Wih-part precomputed as one fp32 matmul.
  - Scoring (sigmoid(last @ Wscore^T) and the alpha/scale blend) per core,
    host concatenates the [8, 2000] outputs.

Column order trick: basket columns are laid out (b//8)*240 + s*8 + (b%8) so
the ReduceScatter hands each core a contiguous, time-major [128, 240] block
of exactly its 8 batches.
"""
import os

import numpy as np

N, E, B, S, D, U = 2000, 3, 64, 30, 128, 128
BS = B * S           # 1920
P = 8                # cores
COLS = N // P        # 250
KT = 16              # k tiles over N
KP = N // KT         # 125
BL = B // P          # 8 local batches
ALPHA = 0.5

_CACHE = {}


def _softmax_row0(w):
    w = np.asarray(w, np.float64)
    m = w - w.max(axis=1, keepdims=True)
    e = np.exp(m)
    return (e / e.sum(axis=1, keepdims=True))[0].astype(np.float32)


def _build(sa, sb, s2, dbg=False):
    import concourse.bacc as bacc
    import concourse.bass as bass
    import concourse.tile as tile
    import concourse.mybir as mybir
    from concourse.alu_op_type import AluOpType
    from contextlib import ExitStack

    f32, f16 = mybir.dt.float32, mybir.dt.float16
    AF = mybir.ActivationFunctionType

    nc = bacc.Bacc("TRN2", target_bir_lowering=False, debug=False, num_devices=P)

    ap_in = nc.dram_tensor("ap", [E, KT, KP, COLS], f16, kind="ExternalInput").ap()
    xt_in = nc.dram_tensor("xt", [KT, KP, BS], f16, kind="ExternalInput").ap()
    xto_in = nc.dram_tensor("xto", [2, KP, BS], f32, kind="ExternalInput").ap()
    sco_in = nc.dram_tensor("sco", [2, KP, 1], f32, kind="ExternalInput").ap()
    lwt_in = nc.dram_tensor("lwt", [2, KP, D], f32, kind="ExternalInput").ap()
    lb_in = nc.dram_tensor("lb", [D, 1], f32, kind="ExternalInput").ap()
    wih_in = nc.dram_tensor("wih", [U, 4 * U], f32, kind="ExternalInput").ap()
    whh_in = nc.dram_tensor("whh", [U, 4 * U], f16, kind="ExternalInput").ap()
    gb_in = nc.dram_tensor("gb", [1, 4 * U], f32, kind="ExternalInput").ap()
    wsc_in = nc.dram_tensor("wsc", [U, N], f32, kind="ExternalInput").ap()
    mp_in = nc.dram_tensor("mp", [U, S * BL], mybir.dt.uint8, kind="ExternalInput").ap()
    nthr_in = nc.dram_tensor("nthr", [KP, 1], f32, kind="ExternalInput").ap()
    wb_in = nc.dram_tensor("wb", [BL, N], f32, kind="ExternalInput").ap()
    h0_in = nc.dram_tensor("h0t", [U, BL], f16, kind="ExternalInput").ap()
    c0_in = nc.dram_tensor("c0t", [U, BL], f32, kind="ExternalInput").ap()
    out_t = nc.dram_tensor("out", [BL, N], f32, kind="ExternalOutput").ap()
    if dbg:
        d_cmb0 = nc.dram_tensor("d_cmb0", [KP, KT * COLS], f16, kind="ExternalOutput").ap()
        d_y1f = nc.dram_tensor("d_y1f", [P, KP, 2, BS], f16, kind="ExternalOutput").ap()
        d_enc0 = nc.dram_tensor("d_enc0", [KP, BS], f32, kind="ExternalOutput").ap()
        d_bkp = nc.dram_tensor("d_bkp", [D, BS], f32, kind="ExternalOutput").ap()
        d_bkc = nc.dram_tensor("d_bkc", [D, S * BL], f32, kind="ExternalOutput").ap()
        d_g1t = nc.dram_tensor("d_g1t", [U, 4, S * BL], f32, kind="ExternalOutput").ap()
        d_last = nc.dram_tensor("d_last", [U, BL], f32, kind="ExternalOutput").ap()
        d_wsc = nc.dram_tensor("d_wsc", [U, N], f32, kind="ExternalOutput").ap()
        d_sig = nc.dram_tensor("d_sig", [BL, N], f32, kind="ExternalOutput").ap()

    mixes = [sa, sb, s2]

    with ExitStack() as ctx:
        tc = ctx.enter_context(tile.TileContext(nc))
        wp = ctx.enter_context(tc.tile_pool(name="w", bufs=1))
        xp = ctx.enter_context(tc.tile_pool(name="x", bufs=1))
        ep = ctx.enter_context(tc.tile_pool(name="e", bufs=1))
        pp = ctx.enter_context(tc.tile_pool(name="ps", bufs=2, space="PSUM"))
        dr = ctx.enter_context(tc.tile_pool(name="dr", bufs=1, space="DRAM"))

        # ---- load A planes; combine into a0s/b0s/a2s (fp16) ----
        apl = [wp.tile([KP, KT * COLS], f16, name=f"apl{e}") for e in range(E)]
        for e in range(E):
            for kt in range(KT):
                nc.sync.dma_start(apl[e][:, kt * COLS:(kt + 1) * COLS], ap_in[e, kt])
        cmb = [wp.tile([KP, KT * COLS], f16, name=f"cmb{w}") for w in range(3)]
        for w in (0, 2, 1):
            mix = mixes[w]
            eng = nc.vector
            eng.tensor_scalar_mul(cmb[w][:], apl[0][:], float(mix[0]))
            eng.scalar_tensor_tensor(
                cmb[w][:], apl[1][:], float(mix[1]), cmb[w][:],
                AluOpType.mult, AluOpType.add)
            eng.scalar_tensor_tensor(
                cmb[w][:], apl[2][:], float(mix[2]), cmb[w][:],
                AluOpType.mult, AluOpType.add)

        # ---- persistent small weights ----
        wih = wp.tile([U, 4 * U], f32, name="wih")
        nc.sync.dma_start(wih[:], wih_in[:])
        whh = wp.tile([U, 4 * U], f16, name="whh")
        nc.sync.dma_start(whh[:], whh_in[:])
        gbt = wp.tile([1, 4 * U], f32, name="gbt")
        nc.sync.dma_start(gbt[:], gb_in[:])
        wsc = wp.tile([U, N], f32, name="wsc")
        nc.sync.dma_start(wsc[:], wsc_in[:])
        mpt = wp.tile([U, S * BL], mybir.dt.uint8, name="mpt")
        nc.sync.dma_start(mpt[:], mp_in[:])
        lbt = wp.tile([D, 1], f32, name="lbt")
        nc.sync.dma_start(lbt[:], lb_in[:])
        ntt = wp.tile([KP, 1], f32, name="ntt")
        nc.sync.dma_start(ntt[:], nthr_in[:])
        scot = [wp.tile([KP, 1], f32, name=f"scot{i}") for i in range(2)]
        for i in range(2):
            nc.sync.dma_start(scot[i][:], sco_in[i])
        lwtt = [wp.tile([KP, D], f32, name=f"lwtt{i}") for i in range(2)]
        for i in range(2):
            nc.sync.dma_start(lwtt[i][:], lwt_in[i])
        wbt = wp.tile([BL, N], f32, name="wbt")
        nc.sync.dma_start(wbt[:], wb_in[:])
        ones = wp.tile([1, S * BL], f32, name="ones")
        nc.vector.memset(ones[:], 1.0)

        # ---- chain: column-halved wavefront so AllGather(half) overlaps ----
        # the other half's matmuls. psum bank = 512 f32 words; matmul
        # outputs must stay inside a bank.
        HALVES = [(0, 1024, [(0, 512), (512, 512)]),
                  (1024, 896, [(1024, 512), (1536, 384)])]

        # resident rhs k-tiles, reused by all three stages
        rt = [xp.tile([KP, BS], f16, name=f"rt{kt}") for kt in range(KT)]
        dma_eng = [nc.sync, nc.scalar]

        def chain_stage(w_idx, loader):
            psA = pp.tile([KP, BS], f32, tag="ps")
            psB = pp.tile([KP, BS], f32, tag="ps")
            pst = [psA, psB]
            for h, (hoff, hsz, banks) in enumerate(HALVES):
                for kt in range(KT):
                    loader(kt, h)
                    for mt in range(2):
                        lhs = cmb[w_idx][:, kt * COLS + mt * KP: kt * COLS + (mt + 1) * KP]
                        for off, sz in banks:
                            nc.tensor.matmul(
                                pst[mt][:, off:off + sz],
                                lhs, rt[kt][:, off:off + sz],
                                start=(kt == 0), stop=(kt == KT - 1))
            return psA, psB

        def evac_ag(psA, psB, nm):
            agos = []
            for h, (hoff, hsz, banks) in enumerate(HALVES):
                yo = ep.tile([KP, 2 * hsz], f16, tag=f"yo{h}", name=f"{nm}y{h}")
                nc.vector.tensor_copy(yo[:, 0:hsz], psA[:, hoff:hoff + hsz])
                nc.vector.tensor_copy(yo[:, hsz:2 * hsz], psB[:, hoff:hoff + hsz])
                agi = dr.tile([KP, 2 * hsz], f16, name=f"{nm}i{h}")
                ago = dr.tile([P, KP, 2, hsz], f16, addr_space="Shared",
                              name=f"{nm}o{h}")
                nc.sync.dma_start(agi[:], yo[:])
                nc.gpsimd.collective_compute(
                    "AllGather", mybir.AluOpType.bypass,
                    replica_groups=[list(range(P))],
                    ins=[agi[:].opt()], outs=[ago[:].opt()])
                agos.append(ago)
            return agos

        def loader_xt(kt, h):
            if h == 0:
                dma_eng[kt % 2].dma_start(rt[kt][:], xt_in[kt])

        def mk_loader(agos):
            def loader(kt, h):
                hoff, hsz, _ = HALVES[h]
                dma_eng[kt % 2].dma_start(
                    rt[kt][:, hoff:hoff + hsz],
                    agos[h][kt // 2, :, kt % 2, :])
            return loader

        # stage 1
        psA, psB = chain_stage(0, loader_xt)
        ag1 = evac_ag(psA, psB, "ag1")
        if dbg:
            nc.sync.dma_start(d_cmb0[:], cmb[0][:])

        # stage 2
        psA, psB = chain_stage(1, mk_loader(ag1))
        ag2 = evac_ag(psA, psB, "ag2")

        # stage 3 -> y3 shard stays in psum
        psA, psB = chain_stage(2, mk_loader(ag2))

        # ---- enc = x*scale + relu(y3 - thr)  (fp32) ----
        enc = [ep.tile([KP, BS], f32, tag=f"enc{i}", name=f"enc{i}") for i in range(2)]
        xto = [ep.tile([KP, BS], f32, tag=f"xto{i}", name=f"xto{i}") for i in range(2)]
        for i in range(2):
            nc.sync.dma_start(xto[i][:], xto_in[i])
        for i, ps in enumerate((psA, psB)):
            nc.scalar.activation(enc[i][:], ps[:], AF.Relu, bias=ntt[:])
            nc.vector.scalar_tensor_tensor(
                enc[i][:], xto[i][:], scot[i][:], enc[i][:],
                AluOpType.mult, AluOpType.add)

        # ---- basket partial: lin_w_shard @ enc  -> [128, 1920] fp32 ----
        ALL_BANKS = [(0, 512), (512, 512), (1024, 512), (1536, 384)]
        psC = pp.tile([D, BS], f32, tag="ps")
        for i in range(2):
            for off, sz in ALL_BANKS:
                nc.tensor.matmul(
                    psC[:, off:off + sz],
                    lwtt[i][:], enc[i][:, off:off + sz],
                    start=(i == 0), stop=(i == 1))
        bkp = ep.tile([D, BS], f32, tag="bkp")
        nc.vector.tensor_copy(bkp[:], psC[:])
        if dbg:
            nc.sync.dma_start(d_enc0[:], enc[0][:])
            nc.sync.dma_start(d_bkp[:], bkp[:])

        rs_i = dr.tile([P, D, S * BL], f32, name="rsi")
        rs_o = dr.tile([D, S * BL], f32, name="rso")
        for r in range(P):
            nc.sync.dma_start(rs_i[r], bkp[:, r * S * BL:(r + 1) * S * BL])
        nc.gpsimd.collective_compute(
            "ReduceScatter", mybir.AluOpType.add,
            replica_groups=[list(range(P))],
            ins=[rs_i[:].opt()], outs=[rs_o[:].opt()])

        # basketT_c = relu(sum + lin_b)
        bkc = wp.tile([D, S * BL], f32, name="bkc")
        nc.sync.dma_start(bkc[:], rs_o[:])
        nc.scalar.activation(bkc[:], bkc[:], AF.Relu, bias=lbt[:])

        # ---- G1T = WihT^T-blocks @ basketT_c + bias (fp32, gate order i,f,o,g) ----
        psD = pp.tile([U, 4 * 512], f32, tag="ps")
        for mg in range(4):
            sl = psD[:, mg * 512: mg * 512 + S * BL]
            nc.tensor.matmul(sl, wih[:, mg * U:(mg + 1) * U], bkc[:],
                             start=True, stop=False)
            nc.tensor.matmul(sl, gbt[:, mg * U:(mg + 1) * U], ones[:],
                             start=False, stop=True)
        g1t = wp.tile([U, 4, S * BL], f32, name="g1t")
        for mg in range(4):
            nc.vector.tensor_copy(g1t[:, mg, :],
                                  psD[:, mg * 512: mg * 512 + S * BL])
        if dbg:
            nc.sync.dma_start(d_bkc[:], bkc[:])
            nc.sync.dma_start(d_g1t[:], g1t[:])

        # ---- LSTM (batch-sharded, transposed layout [U, batch]) ----
        hT = wp.tile([U, BL], f16, name="hT")
        nc.sync.dma_start(hT[:], h0_in[:])
        cT = wp.tile([U, BL], f32, name="cT")
        nc.sync.dma_start(cT[:], c0_in[:])
        lastT = wp.tile([U, BL], f32, name="lastT")
        nc.vector.memset(lastT[:], 0.0)
        gall = wp.tile([U, 4, BL], f32, name="gall")
        sall = wp.tile([U, 4, BL], f32, name="sall")
        tg2 = wp.tile([U, BL], f32, name="tg2")
        tch = wp.tile([U, BL], f32, name="tch")
        h32 = wp.tile([U, BL], f32, name="h32")

        for t in range(S):
            psE = pp.tile([U, 4, BL], f32, tag="ps", name="psE")
            for mg in range(4):
                nc.tensor.matmul(psE[:, mg, :],
                                 whh[:, mg * U:(mg + 1) * U], hT[:],
                                 start=True, stop=True)
            nc.vector.scalar_tensor_tensor(
                gall[:], psE[:], 1.0, g1t[:, :, t * BL:(t + 1) * BL],
                AluOpType.mult, AluOpType.add)
            # gate order i,f,o,g: sigmoid on first 3 blocks, tanh on last
            nc.scalar.activation(sall[:, 0:3, :], gall[:, 0:3, :], AF.Sigmoid)
            nc.scalar.activation(sall[:, 3, :], gall[:, 3, :], AF.Tanh)
            si = sall[:, 0, :]
            sf = sall[:, 1, :]
            so = sall[:, 2, :]
            tg = sall[:, 3, :]
            nc.vector.tensor_tensor(tg2[:], si, tg, AluOpType.mult)
            nc.vector.tensor_tensor(cT[:], sf, cT[:], AluOpType.mult)
            nc.vector.tensor_tensor(cT[:], cT[:], tg2[:], AluOpType.add)
            nc.scalar.activation(tch[:], cT[:], AF.Tanh)
            nc.vector.tensor_tensor(h32[:], so, tch[:], AluOpType.mult)
            nc.vector.tensor_copy(hT[:], h32[:])
            nc.vector.copy_predicated(
                lastT[:], mpt[:, t * BL:(t + 1) * BL], h32[:])

        # ---- scoring ----
        NS = 4
        FS = N // NS  # 500
        psF = pp.tile([BL, 4 * 512], f32, tag="ps", name="psF")
        for b in range(NS):
            nc.tensor.matmul(psF[:, b * 512: b * 512 + FS],
                             lastT[:], wsc[:, b * FS:(b + 1) * FS],
                             start=True, stop=True)
        if dbg:
            nc.sync.dma_start(d_last[:], lastT[:])
        probs = wp.tile([BL, N], f32, name="probs")
        for b in range(NS):
            nc.scalar.activation(probs[:, b * FS:(b + 1) * FS],
                                 psF[:, b * 512: b * 512 + FS], AF.Sigmoid)
        if dbg:
            nc.sync.dma_start(d_wsc[:], wsc[:])
            nc.sync.dma_start(d_sig[:], probs[:])
        nc.vector.tensor_tensor(probs[:], probs[:], wbt[:], AluOpType.mult)
        nc.sync.dma_start(out_t[:], probs[:])

    nc.compile()
    return nc


def kernel(A, seq_len, seqs, h0, c0, W1a, W1b, W2, lin_w, lin_b,
           Wih, Whh, bih, bhh, Wscore, I_B, threshold):
    f32, f16 = np.float32, np.float16
    A = np.asarray(A, f32)
    seqs = np.asarray(seqs, f32)
    seq_len = np.asarray(seq_len).astype(np.int64)
    lin_w = np.asarray(lin_w, f32)
    lin_b = np.asarray(lin_b, f32)
    Wih = np.asarray(Wih, f32)
    Whh = np.asarray(Whh, f32)
    bias = (np.asarray(bih, f32) + np.asarray(bhh, f32))
    Wscore = np.asarray(Wscore, f32)
    scale = np.maximum(np.asarray(I_B, f32), 0.0)
    thr = float(np.asarray(threshold, f32).reshape(-1)[0])

    sa = _softmax_row0(W1a)
    sb = _softmax_row0(W1b)
    s2 = _softmax_row0(W2)

    key = (sa.tobytes(), sb.tobytes(), s2.tobytes())
    if key not in _CACHE:
        _CACHE.clear()
        _CACHE[key] = _build(sa, sb, s2)
    nc = _CACHE[key]

    # column permutation: col(b, s) = (b//BL)*S*BL + s*BL + b%BL
    b_idx = np.arange(B)[:, None]
    s_idx = np.arange(S)[None, :]
    cols_of = ((b_idx // BL) * (S * BL) + s_idx * BL + (b_idx % BL)).reshape(-1)
    x = seqs.reshape(BS, N)
    xTp = np.empty((N, BS), f32)
    xTp[:, cols_of] = x.T
    xt16 = np.ascontiguousarray(xTp.astype(f16).reshape(KT, KP, BS))

    # gate reorder (i, f, o, g)
    gidx = np.r_[0:2 * U, 3 * U:4 * U, 2 * U:3 * U]
    WihT = np.ascontiguousarray(Wih[gidx].T)          # [U, 4U]
    WhhT16 = np.ascontiguousarray(Whh[gidx].T.astype(f16))
    gb = np.ascontiguousarray(bias[gidx][None, :])

    WscoreT = np.ascontiguousarray(Wscore.T)          # [U, N]
    wb_row = ((1.0 - ALPHA) + ALPHA * scale).astype(f32)
    wb = np.ascontiguousarray(np.broadcast_to(wb_row[None, :], (BL, N)))
    lb = np.ascontiguousarray(lin_b.reshape(D, 1))
    nthr = np.full((KP, 1), -thr, f32)
    h0T = np.asarray(h0, f32)[0].T                    # [U, B]
    c0T = np.asarray(c0, f32)[0].T

    in_maps = []
    for c in range(P):
        cl = slice(c * COLS, (c + 1) * COLS)
        apc = np.ascontiguousarray(
            A[:, cl, :].transpose(2, 0, 1).reshape(E, KT, KP, COLS).astype(f16))
        xto = np.ascontiguousarray(xTp[cl].reshape(2, KP, BS))
        sco = np.ascontiguousarray(scale[cl].reshape(2, KP, 1))
        lwt = np.ascontiguousarray(lin_w[:, cl].T.reshape(2, KP, D))
        mp = np.zeros((S * BL,), np.uint8)
        for bl in range(BL):
            t_sel = int(seq_len[c * BL + bl]) - 1
            mp[t_sel * BL + bl] = 1
        mpP = np.ascontiguousarray(np.broadcast_to(mp[None, :], (U, S * BL)))
        in_maps.append({
            "ap": apc, "xt": xt16, "xto": xto, "sco": sco, "lwt": lwt,
            "lb": lb, "wih": WihT, "whh": WhhT16, "gb": gb, "wsc": WscoreT,
            "mp": mpP, "nthr": nthr, "wb": wb,
            "h0t": np.ascontiguousarray(h0T[:, c * BL:(c + 1) * BL].astype(f16)),
            "c0t": np.ascontiguousarray(c0T[:, c * BL:(c + 1) * BL]),
        })

    from concourse.bass_utils import run_bass_kernel_spmd
    trace = bool(os.environ.get("GTN_TRACE"))
    if trace:
        import ntff_shim
        ntff_shim.install()
    res = run_bass_kernel_spmd(nc, in_maps, core_ids=list(range(P)), trace=trace)
    if trace and res.exec_time_ns is not None:
        kernel.last_exec_time_ns = res.exec_time_ns
        kernel.last_trace = res.instructions_and_trace
    predict = np.concatenate([res.results[c]["out"] for c in range(P)], axis=0)
    return predict.astype(f32)


kernel.last_exec_time_ns = None
kernel.last_trace = None


# revision 19
# speedup vs baseline: 1.0605x; 1.0605x over previous
"""GTN-Rec on 8 TRN2 NeuronCores.

Strategy (sharding over the item dim N=2000, 250 columns per core):
  - a0/b0/a2 (softmax-mixed adjacency combos) computed per-core from the
    local A column shard on the vector engine, fp16.
  - Transposed matmul chain y1T/y2T/y3T = (a0|b0|a2)^T-shard @ prev, with
    fp16 AllGather of the [250, 1920] activation shards between stages.
  - enc/lin layer in fp32 (values ~1e7 need precision), ReduceScatter of the
    [128, 1920] basket partial into batch shards (8 batches per core).
  - LSTM tail runs batch-sharded in a transposed [U, batch] layout so h
    needs no per-step transpose; Wih-part precomputed as one fp32 matmul.
  - Scoring (sigmoid(last @ Wscore^T) and the alpha/scale blend) per core,
    host concatenates the [8, 2000] outputs.

Column order trick: basket columns are laid out (b//8)*240 + s*8 + (b%8) so
the ReduceScatter hands each core a contiguous, time-major [128, 240] block
of exactly its 8 batches.
"""
import os

import numpy as np

N, E, B, S, D, U = 2000, 3, 64, 30, 128, 128
BS = B * S           # 1920
P = 8                # cores
COLS = N // P        # 250
KT = 16              # k tiles over N
KP = N // KT         # 125
BL = B // P          # 8 local batches
ALPHA = 0.5

_CACHE = {}


def _softmax_row0(w):
    w = np.asarray(w, np.float64)
    m = w - w.max(axis=1, keepdims=True)
    e = np.exp(m)
    return (e / e.sum(axis=1, keepdims=True))[0].astype(np.float32)


def _build(sa, sb, s2, dbg=False):
    import concourse.bacc as bacc
    import concourse.bass as bass
    import concourse.tile as tile
    import concourse.mybir as mybir
    from concourse.alu_op_type import AluOpType
    from contextlib import ExitStack

    f32, f16 = mybir.dt.float32, mybir.dt.float16
    AF = mybir.ActivationFunctionType

    nc = bacc.Bacc("TRN2", target_bir_lowering=False, debug=False, num_devices=P)

    ap_in = nc.dram_tensor("ap", [E, KT, KP, COLS], f16, kind="ExternalInput").ap()
    xt_in = nc.dram_tensor("xt", [KT, KP, BS], f16, kind="ExternalInput").ap()
    xto_in = nc.dram_tensor("xto", [2, KP, BS], f32, kind="ExternalInput").ap()
    sco_in = nc.dram_tensor("sco", [2, KP, 1], f32, kind="ExternalInput").ap()
    lwt_in = nc.dram_tensor("lwt", [2, KP, D], f32, kind="ExternalInput").ap()
    lb_in = nc.dram_tensor("lb", [D, 1], f32, kind="ExternalInput").ap()
    wih_in = nc.dram_tensor("wih", [U, 4 * U], f32, kind="ExternalInput").ap()
    whh_in = nc.dram_tensor("whh", [U, 4 * U], f16, kind="ExternalInput").ap()
    gb_in = nc.dram_tensor("gb", [1, 4 * U], f32, kind="ExternalInput").ap()
    wsc_in = nc.dram_tensor("wsc", [U, N], f32, kind="ExternalInput").ap()
    mp_in = nc.dram_tensor("mp", [U, S * BL], mybir.dt.uint8, kind="ExternalInput").ap()
    nthr_in = nc.dram_tensor("nthr", [KP, 1], f32, kind="ExternalInput").ap()
    wb_in = nc.dram_tensor("wb", [BL, N], f32, kind="ExternalInput").ap()
    h0_in = nc.dram_tensor("h0t", [U, BL], f16, kind="ExternalInput").ap()
    c0_in = nc.dram_tensor("c0t", [U, BL], f32, kind="ExternalInput").ap()
    out_t = nc.dram_tensor("out", [BL, N], f32, kind="ExternalOutput").ap()
    if dbg:
        d_cmb0 = nc.dram_tensor("d_cmb0", [KP, KT * COLS], f16, kind="ExternalOutput").ap()
        d_y1f = nc.dram_tensor("d_y1f", [P, KP, 2, BS], f16, kind="ExternalOutput").ap()
        d_enc0 = nc.dram_tensor("d_enc0", [KP, BS], f32, kind="ExternalOutput").ap()
        d_bkp = nc.dram_tensor("d_bkp", [D, BS], f32, kind="ExternalOutput").ap()
        d_bkc = nc.dram_tensor("d_bkc", [D, S * BL], f32, kind="ExternalOutput").ap()
        d_g1t = nc.dram_tensor("d_g1t", [U, 4, S * BL], f32, kind="ExternalOutput").ap()
        d_last = nc.dram_tensor("d_last", [U, BL], f32, kind="ExternalOutput").ap()
        d_wsc = nc.dram_tensor("d_wsc", [U, N], f32, kind="ExternalOutput").ap()
        d_sig = nc.dram_tensor("d_sig", [BL, N], f32, kind="ExternalOutput").ap()

    mixes = [sa, sb, s2]

    with ExitStack() as ctx:
        tc = ctx.enter_context(tile.TileContext(nc))
        wp = ctx.enter_context(tc.tile_pool(name="w", bufs=1))
        xp = ctx.enter_context(tc.tile_pool(name="x", bufs=1))
        ep = ctx.enter_context(tc.tile_pool(name="e", bufs=1))
        pp = ctx.enter_context(tc.tile_pool(name="ps", bufs=4, space="PSUM"))
        dr = ctx.enter_context(tc.tile_pool(name="dr", bufs=1, space="DRAM"))

        # ---- load A planes; combine into a0s/b0s/a2s (fp16) ----
        apl = [wp.tile([KP, KT * COLS], f16, name=f"apl{e}") for e in range(E)]
        for e in range(E):
            for kt in range(KT):
                nc.sync.dma_start(apl[e][:, kt * COLS:(kt + 1) * COLS], ap_in[e, kt])
        cmb = [wp.tile([KP, KT * COLS], f16, name=f"cmb{w}") for w in range(3)]
        for w in (0, 2, 1):
            mix = mixes[w]
            eng = nc.vector
            eng.tensor_scalar_mul(cmb[w][:], apl[0][:], float(mix[0]))
            eng.scalar_tensor_tensor(
                cmb[w][:], apl[1][:], float(mix[1]), cmb[w][:],
                AluOpType.mult, AluOpType.add)
            eng.scalar_tensor_tensor(
                cmb[w][:], apl[2][:], float(mix[2]), cmb[w][:],
                AluOpType.mult, AluOpType.add)

        # ---- persistent small weights ----
        wih = wp.tile([U, 4 * U], f32, name="wih")
        nc.sync.dma_start(wih[:], wih_in[:])
        whh = wp.tile([U, 4 * U], f16, name="whh")
        nc.sync.dma_start(whh[:], whh_in[:])
        gbt = wp.tile([1, 4 * U], f32, name="gbt")
        nc.sync.dma_start(gbt[:], gb_in[:])
        wsc = wp.tile([U, N], f32, name="wsc")
        nc.sync.dma_start(wsc[:], wsc_in[:])
        mpt = wp.tile([U, S * BL], mybir.dt.uint8, name="mpt")
        nc.sync.dma_start(mpt[:], mp_in[:])
        lbt = wp.tile([D, 1], f32, name="lbt")
        nc.sync.dma_start(lbt[:], lb_in[:])
        ntt = wp.tile([KP, 1], f32, name="ntt")
        nc.sync.dma_start(ntt[:], nthr_in[:])
        scot = [wp.tile([KP, 1], f32, name=f"scot{i}") for i in range(2)]
        for i in range(2):
            nc.sync.dma_start(scot[i][:], sco_in[i])
        lwtt = [wp.tile([KP, D], f32, name=f"lwtt{i}") for i in range(2)]
        for i in range(2):
            nc.sync.dma_start(lwtt[i][:], lwt_in[i])
        wbt = wp.tile([BL, N], f32, name="wbt")
        nc.sync.dma_start(wbt[:], wb_in[:])
        ones = wp.tile([1, S * BL], f32, name="ones")
        nc.vector.memset(ones[:], 1.0)

        # ---- chain: column-halved wavefront so AllGather(half) overlaps ----
        # the other half's matmuls. psum bank = 512 f32 words; matmul
        # outputs must stay inside a bank.
        HALVES = [(0, 1024, [(0, 512), (512, 512)]),
                  (1024, 896, [(1024, 512), (1536, 384)])]

        # resident rhs k-tiles, reused by all three stages
        rt = [xp.tile([KP, BS], f16, name=f"rt{kt}") for kt in range(KT)]

        def chain_stage(w_idx, loader, nm):
            # one psum tile per (mt, half): evac of a half never waits on
            # the other half's accumulation
            pst = {}
            for h, (hoff, hsz, banks) in enumerate(HALVES):
                for mt in range(2):
                    pst[(mt, h)] = pp.tile([KP, 1024], f32, tag="ps",
                                           name=f"{nm}p{mt}{h}")
            for h, (hoff, hsz, banks) in enumerate(HALVES):
                for kt in range(KT):
                    loader(kt, h)
                    for mt in range(2):
                        lhs = cmb[w_idx][:, kt * COLS + mt * KP: kt * COLS + (mt + 1) * KP]
                        for off, sz in banks:
                            nc.tensor.matmul(
                                pst[(mt, h)][:, off - hoff:off - hoff + sz],
                                lhs, rt[kt][:, off:off + sz],
                                start=(kt == 0), stop=(kt == KT - 1))
            return pst

        def evac_ag(pst, nm):
            agos = []
            for h, (hoff, hsz, banks) in enumerate(HALVES):
                yo = ep.tile([KP, 2 * hsz], f16, tag=f"yo{h}", name=f"{nm}y{h}")
                nc.vector.tensor_copy(yo[:, 0:hsz], pst[(0, h)][:, 0:hsz])
                nc.vector.tensor_copy(yo[:, hsz:2 * hsz], pst[(1, h)][:, 0:hsz])
                agi = dr.tile([KP, 2 * hsz], f16, name=f"{nm}i{h}")
                ago = dr.tile([P, KP, 2, hsz], f16, addr_space="Shared",
                              name=f"{nm}o{h}")
                nc.sync.dma_start(agi[:], yo[:])
                nc.gpsimd.collective_compute(
                    "AllGather", mybir.AluOpType.bypass,
                    replica_groups=[list(range(P))],
                    ins=[agi[:].opt()], outs=[ago[:].opt()])
                agos.append(ago)
            return agos

        def loader_xt(kt, h):
            if h == 0:
                nc.sync.dma_start(rt[kt][:], xt_in[kt])

        def mk_loader(agos):
            def loader(kt, h):
                hoff, hsz, _ = HALVES[h]
                nc.sync.dma_start(
                    rt[kt][:, hoff:hoff + hsz],
                    agos[h][kt // 2, :, kt % 2, :])
            return loader

        # stage 1
        pst = chain_stage(0, loader_xt, "s1")
        ag1 = evac_ag(pst, "ag1")
        if dbg:
            nc.sync.dma_start(d_cmb0[:], cmb[0][:])

        # stage 2
        pst = chain_stage(1, mk_loader(ag1), "s2")
        ag2 = evac_ag(pst, "ag2")

        # stage 3 -> y3 shard stays in psum
        pst3 = chain_stage(2, mk_loader(ag2), "s3")

        # ---- enc = x*scale + relu(y3 - thr)  (fp32) ----
        enc = [ep.tile([KP, BS], f32, tag=f"enc{i}", name=f"enc{i}") for i in range(2)]
        xto = [ep.tile([KP, BS], f32, tag=f"xto{i}", name=f"xto{i}") for i in range(2)]
        for i in range(2):
            nc.sync.dma_start(xto[i][:], xto_in[i])
        for i in range(2):
            for h, (hoff, hsz, banks) in enumerate(HALVES):
                nc.scalar.activation(enc[i][:, hoff:hoff + hsz],
                                     pst3[(i, h)][:, 0:hsz], AF.Relu, bias=ntt[:])
            nc.vector.scalar_tensor_tensor(
                enc[i][:], xto[i][:], scot[i][:], enc[i][:],
                AluOpType.mult, AluOpType.add)

        # ---- basket partial: lin_w_shard @ enc  -> [128, 1920] fp32 ----
        psC = [pp.tile([D, 1024], f32, tag="ps", name=f"psC{h}") for h in range(2)]
        for i in range(2):
            for h, (hoff, hsz, banks) in enumerate(HALVES):
                for off, sz in banks:
                    nc.tensor.matmul(
                        psC[h][:, off - hoff:off - hoff + sz],
                        lwtt[i][:], enc[i][:, off:off + sz],
                        start=(i == 0), stop=(i == 1))
        bkp = ep.tile([D, BS], f32, tag="bkp")
        for h, (hoff, hsz, banks) in enumerate(HALVES):
            nc.vector.tensor_copy(bkp[:, hoff:hoff + hsz], psC[h][:, 0:hsz])
        if dbg:
            nc.sync.dma_start(d_enc0[:], enc[0][:])
            nc.sync.dma_start(d_bkp[:], bkp[:])

        rs_i = dr.tile([P, D, S * BL], f32, name="rsi")
        rs_o = dr.tile([D, S * BL], f32, name="rso")
        for r in range(P):
            nc.sync.dma_start(rs_i[r], bkp[:, r * S * BL:(r + 1) * S * BL])
        nc.gpsimd.collective_compute(
            "ReduceScatter", mybir.AluOpType.add,
            replica_groups=[list(range(P))],
            ins=[rs_i[:].opt()], outs=[rs_o[:].opt()])

        # basketT_c = relu(sum + lin_b)
        bkc = wp.tile([D, S * BL], f32, name="bkc")
        nc.sync.dma_start(bkc[:], rs_o[:])
        nc.scalar.activation(bkc[:], bkc[:], AF.Relu, bias=lbt[:])

        # ---- G1T = WihT^T-blocks @ basketT_c + bias (fp32, gate order i,f,o,g) ----
        psD = [pp.tile([U, 1024], f32, tag="ps", name=f"psD{i}") for i in range(2)]
        for mg in range(4):
            sl = psD[mg // 2][:, (mg % 2) * 512: (mg % 2) * 512 + S * BL]
            nc.tensor.matmul(sl, wih[:, mg * U:(mg + 1) * U], bkc[:],
                             start=True, stop=False)
            nc.tensor.matmul(sl, gbt[:, mg * U:(mg + 1) * U], ones[:],
                             start=False, stop=True)
        g1t = wp.tile([U, 4, S * BL], f32, name="g1t")
        for mg in range(4):
            nc.vector.tensor_copy(
                g1t[:, mg, :],
                psD[mg // 2][:, (mg % 2) * 512: (mg % 2) * 512 + S * BL])
        if dbg:
            nc.sync.dma_start(d_bkc[:], bkc[:])
            nc.sync.dma_start(d_g1t[:], g1t[:])

        # ---- LSTM (batch-sharded, transposed layout [U, batch]) ----
        hT = wp.tile([U, BL], f16, name="hT")
        nc.sync.dma_start(hT[:], h0_in[:])
        cT = wp.tile([U, BL], f32, name="cT")
        nc.sync.dma_start(cT[:], c0_in[:])
        lastT = wp.tile([U, BL], f32, name="lastT")
        nc.vector.memset(lastT[:], 0.0)
        gall = wp.tile([U, 4, BL], f32, name="gall")
        sall = wp.tile([U, 4, BL], f32, name="sall")
        tg2 = wp.tile([U, BL], f32, name="tg2")
        tch = wp.tile([U, BL], f32, name="tch")
        h32 = wp.tile([U, BL], f32, name="h32")

        for t in range(S):
            psE = pp.tile([U, 4, BL], f32, tag="ps", name="psE")
            for mg in range(4):
                nc.tensor.matmul(psE[:, mg, :],
                                 whh[:, mg * U:(mg + 1) * U], hT[:],
                                 start=True, stop=True)
            nc.vector.scalar_tensor_tensor(
                gall[:], psE[:], 1.0, g1t[:, :, t * BL:(t + 1) * BL],
                AluOpType.mult, AluOpType.add)
            # gate order i,f,o,g: sigmoid on first 3 blocks, tanh on last
            nc.scalar.activation(sall[:, 0:3, :], gall[:, 0:3, :], AF.Sigmoid)
            nc.scalar.activation(sall[:, 3, :], gall[:, 3, :], AF.Tanh)
            si = sall[:, 0, :]
            sf = sall[:, 1, :]
            so = sall[:, 2, :]
            tg = sall[:, 3, :]
            nc.vector.tensor_tensor(tg2[:], si, tg, AluOpType.mult)
            nc.vector.tensor_tensor(cT[:], sf, cT[:], AluOpType.mult)
            nc.vector.tensor_tensor(cT[:], cT[:], tg2[:], AluOpType.add)
            nc.scalar.activation(tch[:], cT[:], AF.Tanh)
            nc.vector.tensor_tensor(h32[:], so, tch[:], AluOpType.mult)
            nc.vector.tensor_copy(hT[:], h32[:])
            nc.vector.copy_predicated(
                lastT[:], mpt[:, t * BL:(t + 1) * BL], h32[:])

        # ---- scoring ----
        NS = 4
        FS = N // NS  # 500
        psF = [pp.tile([BL, 1024], f32, tag="ps", name=f"psF{i}") for i in range(2)]
        for b in range(NS):
            nc.tensor.matmul(psF[b // 2][:, (b % 2) * 512: (b % 2) * 512 + FS],
                             lastT[:], wsc[:, b * FS:(b + 1) * FS],
                             start=True, stop=True)
        if dbg:
            nc.sync.dma_start(d_last[:], lastT[:])
        probs = wp.tile([BL, N], f32, name="probs")
        for b in range(NS):
            nc.scalar.activation(probs[:, b * FS:(b + 1) * FS],
                                 psF[b // 2][:, (b % 2) * 512: (b % 2) * 512 + FS],
                                 AF.Sigmoid)
        if dbg:
            nc.sync.dma_start(d_wsc[:], wsc[:])
            nc.sync.dma_start(d_sig[:], probs[:])
        nc.vector.tensor_tensor(probs[:], probs[:], wbt[:], AluOpType.mult)
        nc.sync.dma_start(out_t[:], probs[:])

    nc.compile()
    return nc


def kernel(A, seq_len, seqs, h0, c0, W1a, W1b, W2, lin_w, lin_b,
           Wih, Whh, bih, bhh, Wscore, I_B, threshold):
    f32, f16 = np.float32, np.float16
    A = np.asarray(A, f32)
    seqs = np.asarray(seqs, f32)
    seq_len = np.asarray(seq_len).astype(np.int64)
    lin_w = np.asarray(lin_w, f32)
    lin_b = np.asarray(lin_b, f32)
    Wih = np.asarray(Wih, f32)
    Whh = np.asarray(Whh, f32)
    bias = (np.asarray(bih, f32) + np.asarray(bhh, f32))
    Wscore = np.asarray(Wscore, f32)
    scale = np.maximum(np.asarray(I_B, f32), 0.0)
    thr = float(np.asarray(threshold, f32).reshape(-1)[0])

    sa = _softmax_row0(W1a)
    sb = _softmax_row0(W1b)
    s2 = _softmax_row0(W2)

    key = (sa.tobytes(), sb.tobytes(), s2.tobytes())
    if key not in _CACHE:
        _CACHE.clear()
        _CACHE[key] = _build(sa, sb, s2)
    nc = _CACHE[key]

    # column permutation: col(b, s) = (b//BL)*S*BL + s*BL + b%BL
    b_idx = np.arange(B)[:, None]
    s_idx = np.arange(S)[None, :]
    cols_of = ((b_idx // BL) * (S * BL) + s_idx * BL + (b_idx % BL)).reshape(-1)
    x = seqs.reshape(BS, N)
    xTp = np.empty((N, BS), f32)
    xTp[:, cols_of] = x.T
    xt16 = np.ascontiguousarray(xTp.astype(f16).reshape(KT, KP, BS))

    # gate reorder (i, f, o, g)
    gidx = np.r_[0:2 * U, 3 * U:4 * U, 2 * U:3 * U]
    WihT = np.ascontiguousarray(Wih[gidx].T)          # [U, 4U]
    WhhT16 = np.ascontiguousarray(Whh[gidx].T.astype(f16))
    gb = np.ascontiguousarray(bias[gidx][None, :])

    WscoreT = np.ascontiguousarray(Wscore.T)          # [U, N]
    wb_row = ((1.0 - ALPHA) + ALPHA * scale).astype(f32)
    wb = np.ascontiguousarray(np.broadcast_to(wb_row[None, :], (BL, N)))
    lb = np.ascontiguousarray(lin_b.reshape(D, 1))
    nthr = np.full((KP, 1), -thr, f32)
    h0T = np.asarray(h0, f32)[0].T                    # [U, B]
    c0T = np.asarray(c0, f32)[0].T

    in_maps = []
    for c in range(P):
        cl = slice(c * COLS, (c + 1) * COLS)
        apc = np.ascontiguousarray(
            A[:, cl, :].transpose(2, 0, 1).reshape(E, KT, KP, COLS).astype(f16))
        xto = np.ascontiguousarray(xTp[cl].reshape(2, KP, BS))
        sco = np.ascontiguousarray(scale[cl].reshape(2, KP, 1))
        lwt = np.ascontiguousarray(lin_w[:, cl].T.reshape(2, KP, D))
        mp = np.zeros((S * BL,), np.uint8)
        for bl in range(BL):
            t_sel = int(seq_len[c * BL + bl]) - 1
            mp[t_sel * BL + bl] = 1
        mpP = np.ascontiguousarray(np.broadcast_to(mp[None, :], (U, S * BL)))
        in_maps.append({
            "ap": apc, "xt": xt16, "xto": xto, "sco": sco, "lwt": lwt,
            "lb": lb, "wih": WihT, "whh": WhhT16, "gb": gb, "wsc": WscoreT,
            "mp": mpP, "nthr": nthr, "wb": wb,
            "h0t": np.ascontiguousarray(h0T[:, c * BL:(c + 1) * BL].astype(f16)),
            "c0t": np.ascontiguousarray(c0T[:, c * BL:(c + 1) * BL]),
        })

    from concourse.bass_utils import run_bass_kernel_spmd
    trace = bool(os.environ.get("GTN_TRACE"))
    if trace:
        import ntff_shim
        ntff_shim.install()
    res = run_bass_kernel_spmd(nc, in_maps, core_ids=list(range(P)), trace=trace)
    if trace and res.exec_time_ns is not None:
        kernel.last_exec_time_ns = res.exec_time_ns
        kernel.last_trace = res.instructions_and_trace
    predict = np.concatenate([res.results[c]["out"] for c in range(P)], axis=0)
    return predict.astype(f32)


kernel.last_exec_time_ns = None
kernel.last_trace = None


# revision 21
# speedup vs baseline: 1.1006x; 1.0378x over previous
"""GTN-Rec on 8 TRN2 NeuronCores.

Strategy (sharding over the item dim N=2000, 250 columns per core):
  - a0/b0/a2 (softmax-mixed adjacency combos) computed per-core from the
    local A column shard on the vector engine, fp16.
  - Transposed matmul chain y1T/y2T/y3T = (a0|b0|a2)^T-shard @ prev, with
    fp16 AllGather of the [250, 1920] activation shards between stages.
  - enc/lin layer in fp32 (values ~1e7 need precision), ReduceScatter of the
    [128, 1920] basket partial into batch shards (8 batches per core).
  - LSTM tail runs batch-sharded in a transposed [U, batch] layout so h
    needs no per-step transpose; Wih-part precomputed as one fp32 matmul.
  - Scoring (sigmoid(last @ Wscore^T) and the alpha/scale blend) per core,
    host concatenates the [8, 2000] outputs.

Column order trick: basket columns are laid out (b//8)*240 + s*8 + (b%8) so
the ReduceScatter hands each core a contiguous, time-major [128, 240] block
of exactly its 8 batches.
"""
import os

import numpy as np

N, E, B, S, D, U = 2000, 3, 64, 30, 128, 128
BS = B * S           # 1920
P = 8                # cores
COLS = N // P        # 250
KT = 16              # k tiles over N
KP = N // KT         # 125
BL = B // P          # 8 local batches
ALPHA = 0.5

_CACHE = {}


def _softmax_row0(w):
    w = np.asarray(w, np.float64)
    m = w - w.max(axis=1, keepdims=True)
    e = np.exp(m)
    return (e / e.sum(axis=1, keepdims=True))[0].astype(np.float32)


def _build(sa, sb, s2, dbg=False):
    import concourse.bacc as bacc
    import concourse.bass as bass
    import concourse.tile as tile
    import concourse.mybir as mybir
    from concourse.alu_op_type import AluOpType
    from contextlib import ExitStack

    f32, f16 = mybir.dt.float32, mybir.dt.float16
    AF = mybir.ActivationFunctionType

    nc = bacc.Bacc("TRN2", target_bir_lowering=False, debug=False, num_devices=P)

    ap_in = nc.dram_tensor("ap", [E, KP, KT * COLS], f16, kind="ExternalInput").ap()
    xt_in = nc.dram_tensor("xt", [KT, KP, BS], f16, kind="ExternalInput").ap()
    xto_in = nc.dram_tensor("xto", [2, KP, BS], f32, kind="ExternalInput").ap()
    sco_in = nc.dram_tensor("sco", [2, KP, 1], f32, kind="ExternalInput").ap()
    lwt_in = nc.dram_tensor("lwt", [2, KP, D], f32, kind="ExternalInput").ap()
    lb_in = nc.dram_tensor("lb", [D, 1], f32, kind="ExternalInput").ap()
    wih_in = nc.dram_tensor("wih", [U, 4 * U], f32, kind="ExternalInput").ap()
    whh_in = nc.dram_tensor("whh", [U, 4 * U], f16, kind="ExternalInput").ap()
    gb_in = nc.dram_tensor("gb", [1, 4 * U], f32, kind="ExternalInput").ap()
    wsc_in = nc.dram_tensor("wsc", [U, N], f32, kind="ExternalInput").ap()
    mp_in = nc.dram_tensor("mp", [U, S * BL], mybir.dt.uint8, kind="ExternalInput").ap()
    nthr_in = nc.dram_tensor("nthr", [KP, 1], f32, kind="ExternalInput").ap()
    wb_in = nc.dram_tensor("wb", [BL, N], f32, kind="ExternalInput").ap()
    h0_in = nc.dram_tensor("h0t", [U, BL], f16, kind="ExternalInput").ap()
    c0_in = nc.dram_tensor("c0t", [U, BL], f32, kind="ExternalInput").ap()
    out_t = nc.dram_tensor("out", [BL, N], f32, kind="ExternalOutput").ap()
    if dbg:
        d_cmb0 = nc.dram_tensor("d_cmb0", [KP, KT * COLS], f16, kind="ExternalOutput").ap()
        d_y1f = nc.dram_tensor("d_y1f", [P, KP, 2, BS], f16, kind="ExternalOutput").ap()
        d_enc0 = nc.dram_tensor("d_enc0", [KP, BS], f32, kind="ExternalOutput").ap()
        d_bkp = nc.dram_tensor("d_bkp", [D, BS], f32, kind="ExternalOutput").ap()
        d_bkc = nc.dram_tensor("d_bkc", [D, S * BL], f32, kind="ExternalOutput").ap()
        d_g1t = nc.dram_tensor("d_g1t", [U, 4, S * BL], f32, kind="ExternalOutput").ap()
        d_last = nc.dram_tensor("d_last", [U, BL], f32, kind="ExternalOutput").ap()
        d_wsc = nc.dram_tensor("d_wsc", [U, N], f32, kind="ExternalOutput").ap()
        d_sig = nc.dram_tensor("d_sig", [BL, N], f32, kind="ExternalOutput").ap()

    mixes = [sa, sb, s2]

    with ExitStack() as ctx:
        tc = ctx.enter_context(tile.TileContext(nc))
        wp = ctx.enter_context(tc.tile_pool(name="w", bufs=1))
        xp = ctx.enter_context(tc.tile_pool(name="x", bufs=1))
        ep = ctx.enter_context(tc.tile_pool(name="e", bufs=1))
        pp = ctx.enter_context(tc.tile_pool(name="ps", bufs=4, space="PSUM"))
        dr = ctx.enter_context(tc.tile_pool(name="dr", bufs=1, space="DRAM"))

        # ---- load A planes; combine into a0s/b0s/a2s (fp16) ----
        apl = [wp.tile([KP, KT * COLS], f16, name=f"apl{e}") for e in range(E)]
        dma_rot = [nc.sync, nc.scalar, nc.gpsimd]
        for e in range(E):
            dma_rot[e % 3].dma_start(apl[e][:], ap_in[e])
        cmb = [wp.tile([KP, KT * COLS], f16, name=f"cmb{w}") for w in range(3)]
        for w in (0, 2, 1):
            mix = mixes[w]
            eng = nc.vector
            eng.tensor_scalar_mul(cmb[w][:], apl[0][:], float(mix[0]))
            eng.scalar_tensor_tensor(
                cmb[w][:], apl[1][:], float(mix[1]), cmb[w][:],
                AluOpType.mult, AluOpType.add)
            eng.scalar_tensor_tensor(
                cmb[w][:], apl[2][:], float(mix[2]), cmb[w][:],
                AluOpType.mult, AluOpType.add)

        # ---- persistent small weights ----
        wih = wp.tile([U, 4 * U], f32, name="wih")
        nc.sync.dma_start(wih[:], wih_in[:])
        whh = wp.tile([U, 4 * U], f16, name="whh")
        nc.sync.dma_start(whh[:], whh_in[:])
        gbt = wp.tile([1, 4 * U], f32, name="gbt")
        nc.sync.dma_start(gbt[:], gb_in[:])
        wsc = wp.tile([U, N], f32, name="wsc")
        nc.sync.dma_start(wsc[:], wsc_in[:])
        mpt = wp.tile([U, S * BL], mybir.dt.uint8, name="mpt")
        nc.sync.dma_start(mpt[:], mp_in[:])
        lbt = wp.tile([D, 1], f32, name="lbt")
        nc.sync.dma_start(lbt[:], lb_in[:])
        ntt = wp.tile([KP, 1], f32, name="ntt")
        nc.sync.dma_start(ntt[:], nthr_in[:])
        scot = [wp.tile([KP, 1], f32, name=f"scot{i}") for i in range(2)]
        for i in range(2):
            nc.sync.dma_start(scot[i][:], sco_in[i])
        lwtt = [wp.tile([KP, D], f32, name=f"lwtt{i}") for i in range(2)]
        for i in range(2):
            nc.sync.dma_start(lwtt[i][:], lwt_in[i])
        wbt = wp.tile([BL, N], f32, name="wbt")
        nc.sync.dma_start(wbt[:], wb_in[:])
        ones = wp.tile([1, S * BL], f32, name="ones")
        nc.vector.memset(ones[:], 1.0)

        # ---- chain: column-halved wavefront so AllGather(half) overlaps ----
        # the other half's matmuls. psum bank = 512 f32 words; matmul
        # outputs must stay inside a bank.
        HALVES = [(0, 1024, [(0, 512), (512, 512)]),
                  (1024, 896, [(1024, 512), (1536, 384)])]

        # resident rhs k-tiles, reused by all three stages
        rt = [xp.tile([KP, BS], f16, name=f"rt{kt}") for kt in range(KT)]

        def chain_stage(w_idx, loader, nm):
            # one psum tile per (mt, half): evac of a half never waits on
            # the other half's accumulation
            pst = {}
            for h, (hoff, hsz, banks) in enumerate(HALVES):
                for mt in range(2):
                    pst[(mt, h)] = pp.tile([KP, 1024], f32, tag="ps",
                                           name=f"{nm}p{mt}{h}")
            for h, (hoff, hsz, banks) in enumerate(HALVES):
                for kt in range(KT):
                    loader(kt, h)
                    for mt in range(2):
                        lhs = cmb[w_idx][:, kt * COLS + mt * KP: kt * COLS + (mt + 1) * KP]
                        for off, sz in banks:
                            nc.tensor.matmul(
                                pst[(mt, h)][:, off - hoff:off - hoff + sz],
                                lhs, rt[kt][:, off:off + sz],
                                start=(kt == 0), stop=(kt == KT - 1))
            return pst

        def evac_ag(pst, nm):
            agos = []
            for h, (hoff, hsz, banks) in enumerate(HALVES):
                yo = ep.tile([KP, 2 * hsz], f16, tag=f"yo{h}", name=f"{nm}y{h}")
                nc.vector.tensor_copy(yo[:, 0:hsz], pst[(0, h)][:, 0:hsz])
                nc.vector.tensor_copy(yo[:, hsz:2 * hsz], pst[(1, h)][:, 0:hsz])
                agi = dr.tile([KP, 2 * hsz], f16, name=f"{nm}i{h}")
                ago = dr.tile([P, KP, 2, hsz], f16, addr_space="Shared",
                              name=f"{nm}o{h}")
                nc.sync.dma_start(agi[:], yo[:])
                nc.gpsimd.collective_compute(
                    "AllGather", mybir.AluOpType.bypass,
                    replica_groups=[list(range(P))],
                    ins=[agi[:].opt()], outs=[ago[:].opt()])
                agos.append(ago)
            return agos

        def loader_xt(kt, h):
            if h == 0:
                dma_rot[kt % 3].dma_start(rt[kt][:], xt_in[kt])

        def mk_loader(agos):
            def loader(kt, h):
                hoff, hsz, _ = HALVES[h]
                dma_rot[kt % 3].dma_start(
                    rt[kt][:, hoff:hoff + hsz],
                    agos[h][kt // 2, :, kt % 2, :])
            return loader

        # stage 1
        pst = chain_stage(0, loader_xt, "s1")
        ag1 = evac_ag(pst, "ag1")
        if dbg:
            nc.sync.dma_start(d_cmb0[:], cmb[0][:])

        # stage 2
        pst = chain_stage(1, mk_loader(ag1), "s2")
        ag2 = evac_ag(pst, "ag2")

        # stage 3 -> y3 shard stays in psum
        pst3 = chain_stage(2, mk_loader(ag2), "s3")

        # ---- enc = x*scale + relu(y3 - thr)  (fp32) ----
        enc = [ep.tile([KP, BS], f32, tag=f"enc{i}", name=f"enc{i}") for i in range(2)]
        xto = [ep.tile([KP, BS], f32, tag=f"xto{i}", name=f"xto{i}") for i in range(2)]
        for i in range(2):
            nc.sync.dma_start(xto[i][:], xto_in[i])
        for i in range(2):
            for h, (hoff, hsz, banks) in enumerate(HALVES):
                nc.scalar.activation(enc[i][:, hoff:hoff + hsz],
                                     pst3[(i, h)][:, 0:hsz], AF.Relu, bias=ntt[:])
            nc.vector.scalar_tensor_tensor(
                enc[i][:], xto[i][:], scot[i][:], enc[i][:],
                AluOpType.mult, AluOpType.add)

        # ---- basket partial: lin_w_shard @ enc  -> [128, 1920] fp32 ----
        psC = [pp.tile([D, 1024], f32, tag="ps", name=f"psC{h}") for h in range(2)]
        for i in range(2):
            for h, (hoff, hsz, banks) in enumerate(HALVES):
                for off, sz in banks:
                    nc.tensor.matmul(
                        psC[h][:, off - hoff:off - hoff + sz],
                        lwtt[i][:], enc[i][:, off:off + sz],
                        start=(i == 0), stop=(i == 1))
        bkp = ep.tile([D, BS], f32, tag="bkp")
        for h, (hoff, hsz, banks) in enumerate(HALVES):
            nc.vector.tensor_copy(bkp[:, hoff:hoff + hsz], psC[h][:, 0:hsz])
        if dbg:
            nc.sync.dma_start(d_enc0[:], enc[0][:])
            nc.sync.dma_start(d_bkp[:], bkp[:])

        rs_i = dr.tile([P, D, S * BL], f32, name="rsi")
        rs_o = dr.tile([D, S * BL], f32, name="rso")
        for r in range(P):
            nc.sync.dma_start(rs_i[r], bkp[:, r * S * BL:(r + 1) * S * BL])
        nc.gpsimd.collective_compute(
            "ReduceScatter", mybir.AluOpType.add,
            replica_groups=[list(range(P))],
            ins=[rs_i[:].opt()], outs=[rs_o[:].opt()])

        # basketT_c = relu(sum + lin_b)
        bkc = wp.tile([D, S * BL], f32, name="bkc")
        nc.sync.dma_start(bkc[:], rs_o[:])
        nc.scalar.activation(bkc[:], bkc[:], AF.Relu, bias=lbt[:])

        # ---- G1T = WihT^T-blocks @ basketT_c + bias (fp32, gate order i,f,o,g) ----
        psD = [pp.tile([U, 1024], f32, tag="ps", name=f"psD{i}") for i in range(2)]
        for mg in range(4):
            sl = psD[mg // 2][:, (mg % 2) * 512: (mg % 2) * 512 + S * BL]
            nc.tensor.matmul(sl, wih[:, mg * U:(mg + 1) * U], bkc[:],
                             start=True, stop=False)
            nc.tensor.matmul(sl, gbt[:, mg * U:(mg + 1) * U], ones[:],
                             start=False, stop=True)
        g1t = wp.tile([U, 4, S * BL], f32, name="g1t")
        for mg in range(4):
            nc.vector.tensor_copy(
                g1t[:, mg, :],
                psD[mg // 2][:, (mg % 2) * 512: (mg % 2) * 512 + S * BL])
        if dbg:
            nc.sync.dma_start(d_bkc[:], bkc[:])
            nc.sync.dma_start(d_g1t[:], g1t[:])

        # ---- LSTM (batch-sharded, transposed layout [U, batch]) ----
        hT = wp.tile([U, BL], f16, name="hT")
        nc.sync.dma_start(hT[:], h0_in[:])
        cT = wp.tile([U, BL], f32, name="cT")
        nc.sync.dma_start(cT[:], c0_in[:])
        lastT = wp.tile([U, BL], f32, name="lastT")
        nc.vector.memset(lastT[:], 0.0)
        gall = wp.tile([U, 4, BL], f32, name="gall")
        sall = wp.tile([U, 4, BL], f32, name="sall")
        tg2 = wp.tile([U, BL], f32, name="tg2")
        tch = wp.tile([U, BL], f32, name="tch")
        h32 = wp.tile([U, BL], f32, name="h32")

        for t in range(S):
            psE = pp.tile([U, 4, BL], f32, tag="ps", name="psE")
            for mg in range(4):
                nc.tensor.matmul(psE[:, mg, :],
                                 whh[:, mg * U:(mg + 1) * U], hT[:],
                                 start=True, stop=True)
            nc.vector.scalar_tensor_tensor(
                gall[:], psE[:], 1.0, g1t[:, :, t * BL:(t + 1) * BL],
                AluOpType.mult, AluOpType.add)
            # gate order i,f,o,g: sigmoid on first 3 blocks, tanh on last
            nc.scalar.activation(sall[:, 0:3, :], gall[:, 0:3, :], AF.Sigmoid)
            nc.scalar.activation(sall[:, 3, :], gall[:, 3, :], AF.Tanh)
            si = sall[:, 0, :]
            sf = sall[:, 1, :]
            so = sall[:, 2, :]
            tg = sall[:, 3, :]
            nc.vector.tensor_tensor(tg2[:], si, tg, AluOpType.mult)
            nc.vector.tensor_tensor(cT[:], sf, cT[:], AluOpType.mult)
            nc.vector.tensor_tensor(cT[:], cT[:], tg2[:], AluOpType.add)
            nc.scalar.activation(tch[:], cT[:], AF.Tanh)
            nc.vector.tensor_tensor(h32[:], so, tch[:], AluOpType.mult)
            nc.vector.tensor_copy(hT[:], h32[:])
            nc.vector.copy_predicated(
                lastT[:], mpt[:, t * BL:(t + 1) * BL], h32[:])

        # ---- scoring ----
        NS = 4
        FS = N // NS  # 500
        psF = [pp.tile([BL, 1024], f32, tag="ps", name=f"psF{i}") for i in range(2)]
        for b in range(NS):
            nc.tensor.matmul(psF[b // 2][:, (b % 2) * 512: (b % 2) * 512 + FS],
                             lastT[:], wsc[:, b * FS:(b + 1) * FS],
                             start=True, stop=True)
        if dbg:
            nc.sync.dma_start(d_last[:], lastT[:])
        probs = wp.tile([BL, N], f32, name="probs")
        for b in range(NS):
            nc.scalar.activation(probs[:, b * FS:(b + 1) * FS],
                                 psF[b // 2][:, (b % 2) * 512: (b % 2) * 512 + FS],
                                 AF.Sigmoid)
        if dbg:
            nc.sync.dma_start(d_wsc[:], wsc[:])
            nc.sync.dma_start(d_sig[:], probs[:])
        nc.vector.tensor_tensor(probs[:], probs[:], wbt[:], AluOpType.mult)
        nc.sync.dma_start(out_t[:], probs[:])

    nc.compile()
    return nc


def kernel(A, seq_len, seqs, h0, c0, W1a, W1b, W2, lin_w, lin_b,
           Wih, Whh, bih, bhh, Wscore, I_B, threshold):
    f32, f16 = np.float32, np.float16
    A = np.asarray(A, f32)
    seqs = np.asarray(seqs, f32)
    seq_len = np.asarray(seq_len).astype(np.int64)
    lin_w = np.asarray(lin_w, f32)
    lin_b = np.asarray(lin_b, f32)
    Wih = np.asarray(Wih, f32)
    Whh = np.asarray(Whh, f32)
    bias = (np.asarray(bih, f32) + np.asarray(bhh, f32))
    Wscore = np.asarray(Wscore, f32)
    scale = np.maximum(np.asarray(I_B, f32), 0.0)
    thr = float(np.asarray(threshold, f32).reshape(-1)[0])

    sa = _softmax_row0(W1a)
    sb = _softmax_row0(W1b)
    s2 = _softmax_row0(W2)

    key = (sa.tobytes(), sb.tobytes(), s2.tobytes())
    if key not in _CACHE:
        _CACHE.clear()
        _CACHE[key] = _build(sa, sb, s2)
    nc = _CACHE[key]

    # column permutation: col(b, s) = (b//BL)*S*BL + s*BL + b%BL
    b_idx = np.arange(B)[:, None]
    s_idx = np.arange(S)[None, :]
    cols_of = ((b_idx // BL) * (S * BL) + s_idx * BL + (b_idx % BL)).reshape(-1)
    x = seqs.reshape(BS, N)
    xTp = np.empty((N, BS), f32)
    xTp[:, cols_of] = x.T
    xt16 = np.ascontiguousarray(xTp.astype(f16).reshape(KT, KP, BS))

    # gate reorder (i, f, o, g)
    gidx = np.r_[0:2 * U, 3 * U:4 * U, 2 * U:3 * U]
    WihT = np.ascontiguousarray(Wih[gidx].T)          # [U, 4U]
    WhhT16 = np.ascontiguousarray(Whh[gidx].T.astype(f16))
    gb = np.ascontiguousarray(bias[gidx][None, :])

    WscoreT = np.ascontiguousarray(Wscore.T)          # [U, N]
    wb_row = ((1.0 - ALPHA) + ALPHA * scale).astype(f32)
    wb = np.ascontiguousarray(np.broadcast_to(wb_row[None, :], (BL, N)))
    lb = np.ascontiguousarray(lin_b.reshape(D, 1))
    nthr = np.full((KP, 1), -thr, f32)
    h0T = np.asarray(h0, f32)[0].T                    # [U, B]
    c0T = np.asarray(c0, f32)[0].T

    in_maps = []
    for c in range(P):
        cl = slice(c * COLS, (c + 1) * COLS)
        apc = np.ascontiguousarray(
            A[:, cl, :].transpose(2, 0, 1).reshape(E, KT, KP, COLS)
            .transpose(0, 2, 1, 3).reshape(E, KP, KT * COLS).astype(f16))
        xto = np.ascontiguousarray(xTp[cl].reshape(2, KP, BS))
        sco = np.ascontiguousarray(scale[cl].reshape(2, KP, 1))
        lwt = np.ascontiguousarray(lin_w[:, cl].T.reshape(2, KP, D))
        mp = np.zeros((S * BL,), np.uint8)
        for bl in range(BL):
            t_sel = int(seq_len[c * BL + bl]) - 1
            mp[t_sel * BL + bl] = 1
        mpP = np.ascontiguousarray(np.broadcast_to(mp[None, :], (U, S * BL)))
        in_maps.append({
            "ap": apc, "xt": xt16, "xto": xto, "sco": sco, "lwt": lwt,
            "lb": lb, "wih": WihT, "whh": WhhT16, "gb": gb, "wsc": WscoreT,
            "mp": mpP, "nthr": nthr, "wb": wb,
            "h0t": np.ascontiguousarray(h0T[:, c * BL:(c + 1) * BL].astype(f16)),
            "c0t": np.ascontiguousarray(c0T[:, c * BL:(c + 1) * BL]),
        })

    from concourse.bass_utils import run_bass_kernel_spmd
    trace = bool(os.environ.get("GTN_TRACE"))
    if trace:
        import ntff_shim
        ntff_shim.install()
    res = run_bass_kernel_spmd(nc, in_maps, core_ids=list(range(P)), trace=trace)
    if trace and res.exec_time_ns is not None:
        kernel.last_exec_time_ns = res.exec_time_ns
        kernel.last_trace = res.instructions_and_trace
    predict = np.concatenate([res.results[c]["out"] for c in range(P)], axis=0)
    return predict.astype(f32)


kernel.last_exec_time_ns = None
kernel.last_trace = None


# revision 23
# speedup vs baseline: 1.1965x; 1.0871x over previous
"""GTN-Rec on 8 TRN2 NeuronCores.

Strategy (sharding over the item dim N=2000, 250 columns per core):
  - a0/b0/a2 (softmax-mixed adjacency combos) computed per-core from the
    local A column shard on the vector engine, fp16.
  - Transposed matmul chain y1T/y2T/y3T = (a0|b0|a2)^T-shard @ prev, with
    fp16 AllGather of the [250, 1920] activation shards between stages.
  - enc/lin layer in fp32 (values ~1e7 need precision), ReduceScatter of the
    [128, 1920] basket partial into batch shards (8 batches per core).
  - LSTM tail runs batch-sharded in a transposed [U, batch] layout so h
    needs no per-step transpose; Wih-part precomputed as one fp32 matmul.
  - Scoring (sigmoid(last @ Wscore^T) and the alpha/scale blend) per core,
    host concatenates the [8, 2000] outputs.

Column order trick: basket columns are laid out (b//8)*240 + s*8 + (b%8) so
the ReduceScatter hands each core a contiguous, time-major [128, 240] block
of exactly its 8 batches.
"""
import os

import numpy as np

N, E, B, S, D, U = 2000, 3, 64, 30, 128, 128
BS = B * S           # 1920
P = 8                # cores
COLS = N // P        # 250
KT = 16              # k tiles over N
KP = N // KT         # 125
BL = B // P          # 8 local batches
ALPHA = 0.5

_CACHE = {}


def _softmax_row0(w):
    w = np.asarray(w, np.float64)
    m = w - w.max(axis=1, keepdims=True)
    e = np.exp(m)
    return (e / e.sum(axis=1, keepdims=True))[0].astype(np.float32)


def _build(sa, sb, s2, dbg=False):
    import concourse.bacc as bacc
    import concourse.bass as bass
    import concourse.tile as tile
    import concourse.mybir as mybir
    from concourse.alu_op_type import AluOpType
    from contextlib import ExitStack

    f32, f16 = mybir.dt.float32, mybir.dt.float16
    AF = mybir.ActivationFunctionType

    nc = bacc.Bacc("TRN2", target_bir_lowering=False, debug=False, num_devices=P)

    ap_in = nc.dram_tensor("ap", [E, KP, KT * COLS], f16, kind="ExternalInput").ap()
    xt_in = nc.dram_tensor("xt", [KT, KP, BS], f16, kind="ExternalInput").ap()
    xto_in = nc.dram_tensor("xto", [2, KP, BS], f32, kind="ExternalInput").ap()
    sco_in = nc.dram_tensor("sco", [2, KP, 1], f32, kind="ExternalInput").ap()
    lwt_in = nc.dram_tensor("lwt", [2, KP, D], f32, kind="ExternalInput").ap()
    lb_in = nc.dram_tensor("lb", [D, 1], f32, kind="ExternalInput").ap()
    wih_in = nc.dram_tensor("wih", [U, 4 * U], f32, kind="ExternalInput").ap()
    whh_in = nc.dram_tensor("whh", [U, 4 * U], f16, kind="ExternalInput").ap()
    gb_in = nc.dram_tensor("gb", [1, 4 * U], f32, kind="ExternalInput").ap()
    wsc_in = nc.dram_tensor("wsc", [U, N], f16, kind="ExternalInput").ap()
    mp_in = nc.dram_tensor("mp", [U, S * BL], mybir.dt.uint8, kind="ExternalInput").ap()
    nthr_in = nc.dram_tensor("nthr", [KP, 1], f32, kind="ExternalInput").ap()
    wb_in = nc.dram_tensor("wb", [BL, N], f32, kind="ExternalInput").ap()
    h0_in = nc.dram_tensor("h0t", [U, BL], f16, kind="ExternalInput").ap()
    c0_in = nc.dram_tensor("c0t", [U, BL], f32, kind="ExternalInput").ap()
    out_t = nc.dram_tensor("out", [BL, N], f32, kind="ExternalOutput").ap()
    if dbg:
        d_cmb0 = nc.dram_tensor("d_cmb0", [KP, KT * COLS], f16, kind="ExternalOutput").ap()
        d_y1f = nc.dram_tensor("d_y1f", [P, KP, 2, BS], f16, kind="ExternalOutput").ap()
        d_enc0 = nc.dram_tensor("d_enc0", [KP, BS], f32, kind="ExternalOutput").ap()
        d_bkp = nc.dram_tensor("d_bkp", [D, BS], f32, kind="ExternalOutput").ap()
        d_bkc = nc.dram_tensor("d_bkc", [D, S * BL], f32, kind="ExternalOutput").ap()
        d_g1t = nc.dram_tensor("d_g1t", [U, 4, S * BL], f32, kind="ExternalOutput").ap()
        d_last = nc.dram_tensor("d_last", [U, BL], f16, kind="ExternalOutput").ap()
        d_wsc = nc.dram_tensor("d_wsc", [U, N], f16, kind="ExternalOutput").ap()
        d_sig = nc.dram_tensor("d_sig", [BL, N], f32, kind="ExternalOutput").ap()

    mixes = [sa, sb, s2]

    with ExitStack() as ctx:
        tc = ctx.enter_context(tile.TileContext(nc))
        wp = ctx.enter_context(tc.tile_pool(name="w", bufs=1))
        xp = ctx.enter_context(tc.tile_pool(name="x", bufs=1))
        ep = ctx.enter_context(tc.tile_pool(name="e", bufs=1))
        pp = ctx.enter_context(tc.tile_pool(name="ps", bufs=4, space="PSUM"))
        dr = ctx.enter_context(tc.tile_pool(name="dr", bufs=1, space="DRAM"))

        # ---- load A planes; combine into a0s/b0s/a2s (fp16) ----
        apl = [wp.tile([KP, KT * COLS], f16, name=f"apl{e}") for e in range(E)]
        dma_rot = [nc.sync, nc.scalar, nc.gpsimd]
        for e in range(E):
            dma_rot[e % 3].dma_start(apl[e][:], ap_in[e])
        cmb = [wp.tile([KP, KT * COLS], f16, name=f"cmb{w}") for w in range(3)]
        CW = KT * COLS // 4
        for w in (0, 1, 2):
            mix = mixes[w]
            for ci in range(4):
                cs = slice(ci * CW, (ci + 1) * CW)
                nc.vector.tensor_scalar_mul(cmb[w][cs := (slice(None), cs)[1]] if False else cmb[w][:, ci * CW:(ci + 1) * CW], apl[0][:, ci * CW:(ci + 1) * CW], float(mix[0]))
                nc.vector.scalar_tensor_tensor(
                    cmb[w][:, ci * CW:(ci + 1) * CW], apl[1][:, ci * CW:(ci + 1) * CW], float(mix[1]),
                    cmb[w][:, ci * CW:(ci + 1) * CW], AluOpType.mult, AluOpType.add)
                nc.vector.scalar_tensor_tensor(
                    cmb[w][:, ci * CW:(ci + 1) * CW], apl[2][:, ci * CW:(ci + 1) * CW], float(mix[2]),
                    cmb[w][:, ci * CW:(ci + 1) * CW], AluOpType.mult, AluOpType.add)

        # ---- persistent small weights ----
        wih = wp.tile([U, 4 * U], f32, name="wih")
        nc.sync.dma_start(wih[:], wih_in[:])
        whh = wp.tile([U, 4 * U], f16, name="whh")
        nc.sync.dma_start(whh[:], whh_in[:])
        gbt = wp.tile([1, 4 * U], f32, name="gbt")
        nc.sync.dma_start(gbt[:], gb_in[:])
        wsc = wp.tile([U, N], f16, name="wsc")
        nc.sync.dma_start(wsc[:], wsc_in[:])
        mpt = wp.tile([U, S * BL], mybir.dt.uint8, name="mpt")
        nc.sync.dma_start(mpt[:], mp_in[:])
        lbt = wp.tile([D, 1], f32, name="lbt")
        nc.sync.dma_start(lbt[:], lb_in[:])
        ntt = wp.tile([KP, 1], f32, name="ntt")
        nc.sync.dma_start(ntt[:], nthr_in[:])
        scot = [wp.tile([KP, 1], f32, name=f"scot{i}") for i in range(2)]
        for i in range(2):
            nc.sync.dma_start(scot[i][:], sco_in[i])
        lwtt = [wp.tile([KP, D], f32, name=f"lwtt{i}") for i in range(2)]
        for i in range(2):
            nc.sync.dma_start(lwtt[i][:], lwt_in[i])
        wbt = wp.tile([BL, N], f32, name="wbt")
        nc.sync.dma_start(wbt[:], wb_in[:])
        ones = wp.tile([1, S * BL], f32, name="ones")
        nc.vector.memset(ones[:], 1.0)

        # ---- chain: column-halved wavefront so AllGather(half) overlaps ----
        # the other half's matmuls. psum bank = 512 f32 words; matmul
        # outputs must stay inside a bank.
        HALVES = [(0, 1024, [(0, 512), (512, 512)]),
                  (1024, 896, [(1024, 512), (1536, 384)])]

        # resident rhs k-tiles, reused by all three stages
        rt = [xp.tile([KP, BS], f16, name=f"rt{kt}") for kt in range(KT)]

        def chain_stage(w_idx, loader, nm):
            # one psum tile per (mt, half): evac of a half never waits on
            # the other half's accumulation
            pst = {}
            for h, (hoff, hsz, banks) in enumerate(HALVES):
                for mt in range(2):
                    pst[(mt, h)] = pp.tile([KP, 1024], f32, tag="ps",
                                           name=f"{nm}p{mt}{h}")
            for h, (hoff, hsz, banks) in enumerate(HALVES):
                for kt in range(KT):
                    loader(kt, h)
                    for mt in range(2):
                        lhs = cmb[w_idx][:, kt * COLS + mt * KP: kt * COLS + (mt + 1) * KP]
                        for off, sz in banks:
                            nc.tensor.matmul(
                                pst[(mt, h)][:, off - hoff:off - hoff + sz],
                                lhs, rt[kt][:, off:off + sz],
                                start=(kt == 0), stop=(kt == KT - 1))
            return pst

        def evac_ag(pst, nm):
            agos = []
            for h, (hoff, hsz, banks) in enumerate(HALVES):
                yo = ep.tile([KP, 2 * hsz], f16, tag=f"yo{h}", name=f"{nm}y{h}")
                nc.vector.tensor_copy(yo[:, 0:hsz], pst[(0, h)][:, 0:hsz])
                nc.vector.tensor_copy(yo[:, hsz:2 * hsz], pst[(1, h)][:, 0:hsz])
                agi = dr.tile([KP, 2 * hsz], f16, name=f"{nm}i{h}")
                ago = dr.tile([P, KP, 2, hsz], f16, addr_space="Shared",
                              name=f"{nm}o{h}")
                nc.sync.dma_start(agi[:], yo[:])
                nc.gpsimd.collective_compute(
                    "AllGather", mybir.AluOpType.bypass,
                    replica_groups=[list(range(P))],
                    ins=[agi[:].opt()], outs=[ago[:].opt()])
                agos.append(ago)
            return agos

        def loader_xt(kt, h):
            if h == 0:
                dma_rot[kt % 3].dma_start(rt[kt][:], xt_in[kt])

        def mk_loader(agos):
            def loader(kt, h):
                hoff, hsz, _ = HALVES[h]
                dma_rot[kt % 3].dma_start(
                    rt[kt][:, hoff:hoff + hsz],
                    agos[h][kt // 2, :, kt % 2, :])
            return loader

        # stage 1
        pst = chain_stage(0, loader_xt, "s1")
        ag1 = evac_ag(pst, "ag1")
        if dbg:
            nc.sync.dma_start(d_cmb0[:], cmb[0][:])

        # stage 2
        pst = chain_stage(1, mk_loader(ag1), "s2")
        ag2 = evac_ag(pst, "ag2")

        # stage 3 -> y3 shard stays in psum
        pst3 = chain_stage(2, mk_loader(ag2), "s3")

        # ---- enc = x*scale + relu(y3 - thr)  (fp32) ----
        enc = [ep.tile([KP, BS], f32, tag=f"enc{i}", name=f"enc{i}") for i in range(2)]
        xto = [ep.tile([KP, BS], f32, tag=f"xto{i}", name=f"xto{i}") for i in range(2)]
        for i in range(2):
            nc.sync.dma_start(xto[i][:], xto_in[i])
        for i in range(2):
            for h, (hoff, hsz, banks) in enumerate(HALVES):
                nc.scalar.activation(enc[i][:, hoff:hoff + hsz],
                                     pst3[(i, h)][:, 0:hsz], AF.Relu, bias=ntt[:])
            for h, (hoff, hsz, banks) in enumerate(HALVES):
                nc.vector.scalar_tensor_tensor(
                    enc[i][:, hoff:hoff + hsz], xto[i][:, hoff:hoff + hsz],
                    scot[i][:], enc[i][:, hoff:hoff + hsz],
                    AluOpType.mult, AluOpType.add)

        # ---- basket partial: lin_w_shard @ enc  -> [128, 1920] fp32 ----
        psC = [pp.tile([D, 1024], f32, tag="ps", name=f"psC{h}") for h in range(2)]
        for i in range(2):
            for h, (hoff, hsz, banks) in enumerate(HALVES):
                for off, sz in banks:
                    nc.tensor.matmul(
                        psC[h][:, off - hoff:off - hoff + sz],
                        lwtt[i][:], enc[i][:, off:off + sz],
                        start=(i == 0), stop=(i == 1))
        bkp = ep.tile([D, BS], f32, tag="bkp")
        for h, (hoff, hsz, banks) in enumerate(HALVES):
            nc.vector.tensor_copy(bkp[:, hoff:hoff + hsz], psC[h][:, 0:hsz])
        if dbg:
            nc.sync.dma_start(d_enc0[:], enc[0][:])
            nc.sync.dma_start(d_bkp[:], bkp[:])

        rs_i = dr.tile([P, D, S * BL], f32, name="rsi")
        rs_o = dr.tile([D, S * BL], f32, name="rso")
        for r in range(P):
            nc.sync.dma_start(rs_i[r], bkp[:, r * S * BL:(r + 1) * S * BL])
        nc.gpsimd.collective_compute(
            "ReduceScatter", mybir.AluOpType.add,
            replica_groups=[list(range(P))],
            ins=[rs_i[:].opt()], outs=[rs_o[:].opt()])

        # basketT_c = relu(sum + lin_b)
        bkc = wp.tile([D, S * BL], f32, name="bkc")
        nc.sync.dma_start(bkc[:], rs_o[:])
        nc.scalar.activation(bkc[:], bkc[:], AF.Relu, bias=lbt[:])

        # ---- G1T = WihT^T-blocks @ basketT_c + bias (fp32, gate order i,f,o,g) ----
        psD = [pp.tile([U, 1024], f32, tag="ps", name=f"psD{i}") for i in range(2)]
        for mg in range(4):
            sl = psD[mg // 2][:, (mg % 2) * 512: (mg % 2) * 512 + S * BL]
            nc.tensor.matmul(sl, wih[:, mg * U:(mg + 1) * U], bkc[:],
                             start=True, stop=False)
            nc.tensor.matmul(sl, gbt[:, mg * U:(mg + 1) * U], ones[:],
                             start=False, stop=True)
        g1t = wp.tile([U, 4, S * BL], f32, name="g1t")
        for mg in range(4):
            nc.vector.tensor_copy(
                g1t[:, mg, :],
                psD[mg // 2][:, (mg % 2) * 512: (mg % 2) * 512 + S * BL])
        if dbg:
            nc.sync.dma_start(d_bkc[:], bkc[:])
            nc.sync.dma_start(d_g1t[:], g1t[:])

        # ---- LSTM (batch-sharded, transposed layout [U, batch]) ----
        hT = wp.tile([U, BL], f16, name="hT")
        nc.sync.dma_start(hT[:], h0_in[:])
        cT = wp.tile([U, BL], f32, name="cT")
        nc.sync.dma_start(cT[:], c0_in[:])
        lastT = wp.tile([U, BL], f16, name="lastT")
        nc.vector.memset(lastT[:], 0.0)
        gall = wp.tile([U, 4, BL], f32, name="gall")
        sall = wp.tile([U, 4, BL], f32, name="sall")
        tg2 = wp.tile([U, BL], f32, name="tg2")
        tch = wp.tile([U, BL], f32, name="tch")

        GW = BL // 2
        for t in range(S):
            psE = pp.tile([U, 4, BL], f32, tag="ps", name="psE")
            for g in range(2):
                gs = slice(g * GW, (g + 1) * GW)
                ms = slice(t * BL + g * GW, t * BL + (g + 1) * GW)
                for mg in range(4):
                    nc.tensor.matmul(psE[:, mg, gs],
                                     whh[:, mg * U:(mg + 1) * U], hT[:, gs],
                                     start=True, stop=True)
                nc.vector.scalar_tensor_tensor(
                    gall[:, :, gs], psE[:, :, gs], 1.0, g1t[:, :, ms],
                    AluOpType.mult, AluOpType.add)
                # gate order i,f,o,g: sigmoid first 3 blocks, tanh on last
                nc.scalar.activation(sall[:, 0:3, gs], gall[:, 0:3, gs], AF.Sigmoid)
                nc.scalar.activation(sall[:, 3, gs], gall[:, 3, gs], AF.Tanh)
                si = sall[:, 0, gs]
                sf = sall[:, 1, gs]
                so = sall[:, 2, gs]
                tg = sall[:, 3, gs]
                nc.vector.tensor_tensor(tg2[:, gs], si, tg, AluOpType.mult)
                nc.vector.tensor_tensor(cT[:, gs], sf, cT[:, gs], AluOpType.mult)
                nc.vector.tensor_tensor(cT[:, gs], cT[:, gs], tg2[:, gs], AluOpType.add)
                nc.scalar.activation(tch[:, gs], cT[:, gs], AF.Tanh)
                nc.vector.tensor_tensor(hT[:, gs], so, tch[:, gs], AluOpType.mult)
                nc.vector.copy_predicated(lastT[:, gs], mpt[:, ms], hT[:, gs])

        # ---- scoring ----
        NS = 4
        FS = N // NS  # 500
        psF = [pp.tile([BL, 1024], f32, tag="ps", name=f"psF{i}") for i in range(2)]
        for b in range(NS):
            nc.tensor.matmul(psF[b // 2][:, (b % 2) * 512: (b % 2) * 512 + FS],
                             lastT[:], wsc[:, b * FS:(b + 1) * FS],
                             start=True, stop=True)
        if dbg:
            nc.sync.dma_start(d_last[:], lastT[:])
        probs = wp.tile([BL, N], f32, name="probs")
        for b in range(NS):
            nc.scalar.activation(probs[:, b * FS:(b + 1) * FS],
                                 psF[b // 2][:, (b % 2) * 512: (b % 2) * 512 + FS],
                                 AF.Sigmoid)
        if dbg:
            nc.sync.dma_start(d_wsc[:], wsc[:])
            nc.sync.dma_start(d_sig[:], probs[:])
        nc.vector.tensor_tensor(probs[:], probs[:], wbt[:], AluOpType.mult)
        nc.sync.dma_start(out_t[:], probs[:])

    nc.compile()
    return nc


def kernel(A, seq_len, seqs, h0, c0, W1a, W1b, W2, lin_w, lin_b,
           Wih, Whh, bih, bhh, Wscore, I_B, threshold):
    f32, f16 = np.float32, np.float16
    A = np.asarray(A, f32)
    seqs = np.asarray(seqs, f32)
    seq_len = np.asarray(seq_len).astype(np.int64)
    lin_w = np.asarray(lin_w, f32)
    lin_b = np.asarray(lin_b, f32)
    Wih = np.asarray(Wih, f32)
    Whh = np.asarray(Whh, f32)
    bias = (np.asarray(bih, f32) + np.asarray(bhh, f32))
    Wscore = np.asarray(Wscore, f32)
    scale = np.maximum(np.asarray(I_B, f32), 0.0)
    thr = float(np.asarray(threshold, f32).reshape(-1)[0])

    sa = _softmax_row0(W1a)
    sb = _softmax_row0(W1b)
    s2 = _softmax_row0(W2)

    key = (sa.tobytes(), sb.tobytes(), s2.tobytes())
    if key not in _CACHE:
        _CACHE.clear()
        _CACHE[key] = _build(sa, sb, s2)
    nc = _CACHE[key]

    # column permutation: col(b, s) = (b//BL)*S*BL + s*BL + b%BL
    b_idx = np.arange(B)[:, None]
    s_idx = np.arange(S)[None, :]
    cols_of = ((b_idx // BL) * (S * BL) + s_idx * BL + (b_idx % BL)).reshape(-1)
    x = seqs.reshape(BS, N)
    xTp = np.empty((N, BS), f32)
    xTp[:, cols_of] = x.T
    xt16 = np.ascontiguousarray(xTp.astype(f16).reshape(KT, KP, BS))

    # gate reorder (i, f, o, g)
    gidx = np.r_[0:2 * U, 3 * U:4 * U, 2 * U:3 * U]
    WihT = np.ascontiguousarray(Wih[gidx].T)          # [U, 4U]
    WhhT16 = np.ascontiguousarray(Whh[gidx].T.astype(f16))
    gb = np.ascontiguousarray(bias[gidx][None, :])

    WscoreT = np.ascontiguousarray(Wscore.T.astype(f16))  # [U, N]
    wb_row = ((1.0 - ALPHA) + ALPHA * scale).astype(f32)
    wb = np.ascontiguousarray(np.broadcast_to(wb_row[None, :], (BL, N)))
    lb = np.ascontiguousarray(lin_b.reshape(D, 1))
    nthr = np.full((KP, 1), -thr, f32)
    h0T = np.asarray(h0, f32)[0].T                    # [U, B]
    c0T = np.asarray(c0, f32)[0].T

    in_maps = []
    for c in range(P):
        cl = slice(c * COLS, (c + 1) * COLS)
        apc = np.ascontiguousarray(
            A[:, cl, :].transpose(2, 0, 1).reshape(E, KT, KP, COLS)
            .transpose(0, 2, 1, 3).reshape(E, KP, KT * COLS).astype(f16))
        xto = np.ascontiguousarray(xTp[cl].reshape(2, KP, BS))
        sco = np.ascontiguousarray(scale[cl].reshape(2, KP, 1))
        lwt = np.ascontiguousarray(lin_w[:, cl].T.reshape(2, KP, D))
        mp = np.zeros((S * BL,), np.uint8)
        for bl in range(BL):
            t_sel = int(seq_len[c * BL + bl]) - 1
            mp[t_sel * BL + bl] = 1
        mpP = np.ascontiguousarray(np.broadcast_to(mp[None, :], (U, S * BL)))
        in_maps.append({
            "ap": apc, "xt": xt16, "xto": xto, "sco": sco, "lwt": lwt,
            "lb": lb, "wih": WihT, "whh": WhhT16, "gb": gb, "wsc": WscoreT,
            "mp": mpP, "nthr": nthr, "wb": wb,
            "h0t": np.ascontiguousarray(h0T[:, c * BL:(c + 1) * BL].astype(f16)),
            "c0t": np.ascontiguousarray(c0T[:, c * BL:(c + 1) * BL]),
        })

    from concourse.bass_utils import run_bass_kernel_spmd
    trace = bool(os.environ.get("GTN_TRACE"))
    if trace:
        import ntff_shim
        ntff_shim.install()
    res = run_bass_kernel_spmd(nc, in_maps, core_ids=list(range(P)), trace=trace)
    if trace and res.exec_time_ns is not None:
        kernel.last_exec_time_ns = res.exec_time_ns
        kernel.last_trace = res.instructions_and_trace
    predict = np.concatenate([res.results[c]["out"] for c in range(P)], axis=0)
    return predict.astype(f32)


kernel.last_exec_time_ns = None
kernel.last_trace = None
